# revision 4
# baseline (speedup 1.0000x reference)
"""Trainium2 Bass kernel for nn_AttentionModulatedOrdinalEmbedding.

Contract: kernel(**inputs) takes the FULL (unsharded) inputs from
setup_inputs() and returns the FULL (B, S, EMB) float32 output.
Internally shards batch-parallel across 8 NeuronCores (4 batches/core),
runs one SPMD Bass kernel, and concatenates the per-core outputs.

Hardcoded problem shape: B=32, S=512, N_Q=1024, N_CATS=4, EMB=64,
ATTN=32, HEADS=4 (head_dim 8).
"""

import os
import sys
from contextlib import ExitStack

import numpy as np

for _p in ("/opt/trn_rl_repo", "/root/.axon_site/_ro/trn_rl_repo"):
    if os.path.isdir(_p) and _p not in sys.path:
        sys.path.append(_p)

import ml_dtypes  # noqa: E402

import concourse.bass as bass  # noqa: E402
import concourse.tile as tile  # noqa: E402
from concourse import bacc, mybir  # noqa: E402
from concourse.bass import IndirectOffsetOnAxis  # noqa: E402
from concourse.bass_utils import run_bass_kernel_spmd  # noqa: E402
from concourse.masks import make_identity  # noqa: E402

BF16 = ml_dtypes.bfloat16
F32 = mybir.dt.float32
BF = mybir.dt.bfloat16
I32 = mybir.dt.int32
ALU = mybir.AluOpType
ACTF = mybir.ActivationFunctionType

B, S, EMB, ATTN, HEADS, HD, C, Q = 32, 512, 64, 32, 4, 8, 4, 1024
NCORES = 8
NB = B // NCORES          # batches per core = 4
NJ = NB * (S // 128)      # token tiles per core = 16
SCALE = 1.0 / np.sqrt(HD)


def build_kernel(nc: bacc.Bacc, tc: tile.TileContext, io: dict):
    """Emit the per-core program. io maps names -> DRAM APs."""
    ctx = ExitStack()
    with ctx:
        _build(nc, tc, ctx, io)


def _build(nc, tc, ctx, io):
    const = ctx.enter_context(tc.tile_pool(name="const", bufs=1))
    sb = ctx.enter_context(tc.tile_pool(name="sb", bufs=2))
    expp = ctx.enter_context(tc.tile_pool(name="expp", bufs=16))
    vsp = ctx.enter_context(tc.tile_pool(name="vsp", bufs=8))
    gp = ctx.enter_context(tc.tile_pool(name="gp", bufs=16))
    big = ctx.enter_context(tc.tile_pool(name="big", bufs=1))
    ps_scores = ctx.enter_context(tc.tile_pool(name="ps_scores", bufs=2, space="PSUM"))
    ps_av = ctx.enter_context(tc.tile_pool(name="ps_av", bufs=1, space="PSUM"))
    ps_sum = ctx.enter_context(tc.tile_pool(name="ps_sum", bufs=1, space="PSUM"))
    ps_misc = ctx.enter_context(tc.tile_pool(name="ps_misc", bufs=2, space="PSUM"))

    # ---------------- constants / weights into SBUF ----------------
    def load_const(name, part, free, dtype):
        t = const.tile([part, free], dtype, tag=name)
        nc.sync.dma_start(out=t[:, :], in_=io[name][:, :])
        return t

    wctxT = load_const("wctxT", EMB, ATTN, BF)          # (64,32)
    bctx = load_const("bctx", ATTN, 1, F32)             # (32,1)
    wq_sp = load_const("wq_sp", ATTN, 128, BF)          # (32,128) head h at cols 32h..32h+8, 0 pad
    wk_sp = load_const("wk_sp", ATTN, 128, BF)
    bq_sp = load_const("bq_sp", 128, 1, F32)
    bk_sp = load_const("bk_sp", 128, 1, F32)
    wvT_aug = load_const("wvT_aug", ATTN + 1, ATTN, BF)  # (33,32) last row = b_v
    wout_sp = load_const("wout_sp", 128, ATTN, BF)      # (128,32) spread layout
    bout = load_const("bout", ATTN, 1, F32)
    wsupT_aug = load_const("wsupT_aug", ATTN + 1, C, BF)  # (33,4) last row = b_sup
    bemb_bc = load_const("bemb_bc", 128, EMB, F32)      # (128,64) replicated
    temp = load_const("temp", 1, HEADS, F32)            # (1,4)
    qidx = load_const("qidx", 128, NJ, I32)             # (128,16) token-tiled
    rdat = load_const("rdat", 128, NJ, I32)

    ident = const.tile([128, 128], BF, tag="ident")
    make_identity(nc, ident[:, :])
    ones1 = const.tile([1, 128], F32, tag="ones1")
    nc.vector.memset(ones1[:, :], 1.0)
    ones_bf = const.tile([128, ATTN], BF, tag="ones_bf")
    nc.vector.memset(ones_bf[:, :], 1.0)

    # iota over cats: (128,16) value = i % 4  (i = 4h + c)
    iota_i = const.tile([128, HEADS * C], I32, tag="iota_i")
    nc.gpsimd.iota(
        iota_i[:, :].rearrange("p (h c) -> p h c", c=C),
        pattern=[[0, HEADS], [1, C]],
        channel_multiplier=0,
    )
    kkf = const.tile([128, HEADS * C], F32, tag="kkf")
    nc.vector.tensor_copy(kkf[:, :], iota_i[:, :])

    # 1/T pattern (1,16) then broadcast to (128,16) via PE
    recipT = const.tile([1, HEADS], F32, tag="recipT")
    nc.vector.reciprocal(recipT[:, :], temp[:, :])
    recipT16 = const.tile([1, HEADS * C], F32, tag="recipT16")
    nc.vector.tensor_copy(
        recipT16[:, :].rearrange("p (h c) -> p h c", c=C),
        recipT[:, :].to_broadcast([1, HEADS, C]),
    )
    rt_ps = ps_misc.tile([128, HEADS * C], F32, tag="misc")
    nc.tensor.matmul(rt_ps[:, :], ones1[:, :], recipT16[:, :], start=True, stop=True)
    rt_bc = const.tile([128, HEADS * C], F32, tag="rt_bc")
    nc.vector.tensor_copy(rt_bc[:, :], rt_ps[:, :])

    # ---------------- sharpened (ordinal softmax) path ----------------
    # tokens: partition p, tile j (j = 4*b + cc); free layout i = 4h + c
    rdf = const.tile([128, NJ], F32, tag="rdf")
    nc.vector.tensor_copy(rdf[:, :], rdat[:, :])
    dmat = big.tile([128, NJ * HEADS * C], F32, tag="dmat")
    d3 = dmat[:, :].rearrange("p (j i) -> p j i", i=HEADS * C)
    nc.vector.tensor_tensor(
        d3,
        kkf[:, None, :].to_broadcast([128, NJ, HEADS * C]),
        rdf[:, :, None].to_broadcast([128, NJ, HEADS * C]),
        op=ALU.subtract,
    )
    # a = -|d|/3   (|d| = max(d, -d))
    ndmat = big.tile([128, NJ * HEADS * C], F32, tag="ndmat")
    nc.vector.tensor_scalar_mul(ndmat[:, :], dmat[:, :], -1.0)
    nc.vector.tensor_tensor(dmat[:, :], dmat[:, :], ndmat[:, :], op=ALU.max)
    nc.vector.tensor_scalar_mul(dmat[:, :], dmat[:, :], -1.0 / (C - 1))
    # bw = relu(1 + a)
    nc.scalar.activation(dmat[:, :], dmat[:, :], ACTF.Relu, bias=1.0, scale=1.0)
    # e_in = bw * (1/T_h)
    nc.vector.tensor_tensor(
        d3,
        d3,
        rt_bc[:, None, :].to_broadcast([128, NJ, HEADS * C]),
        op=ALU.mult,
    )
    nc.scalar.activation(dmat[:, :], dmat[:, :], ACTF.Exp)
    sums2 = big.tile([128, NJ * HEADS], F32, tag="sums2")
    nc.vector.tensor_reduce(
        sums2[:, :],
        dmat[:, :].rearrange("p (a c) -> p a c", c=C),
        axis=mybir.AxisListType.X,
        op=ALU.add,
    )
    r2 = big.tile([128, NJ * HEADS], F32, tag="r2")
    nc.vector.reciprocal(r2[:, :], sums2[:, :])
    p2 = big.tile([128, NJ * HEADS * C], F32, tag="p2")
    nc.vector.scalar_tensor_tensor(
        p2[:, :].rearrange("p (a c) -> p a c", c=C),
        dmat[:, :].rearrange("p (a c) -> p a c", c=C),
        0.125,
        r2[:, :, None].to_broadcast([128, NJ * HEADS, C]),
        op0=ALU.mult,
        op1=ALU.mult,
    )
    sharp = big.tile([128, NJ * C], F32, tag="sharp")
    nc.vector.tensor_reduce(
        sharp[:, :].rearrange("p (j c) -> p j c", c=C),
        p2[:, :].rearrange("p (j h c) -> p j c h", h=HEADS, c=C),
        axis=mybir.AxisListType.X,
        op=ALU.add,
    )

    # ---------------- gathers (independent of attention) ----------------
    gtiles = []
    for j in range(NJ):
        g = gp.tile([128, C * EMB], F32, tag="g")
        nc.gpsimd.indirect_dma_start(
            out=g[:, :],
            out_offset=None,
            in_=io["w3T"][:, :],
            in_offset=IndirectOffsetOnAxis(ap=qidx[:, j : j + 1], axis=0),
        )
        gtiles.append(g)

    # ---------------- context embedding load (cast to bf16) ----------------
    ce = big.tile([128, NJ * EMB], BF, tag="ce")
    nc.gpsimd.dma_start(
        out=ce[:, :].rearrange("p (b cc e) -> p b cc e", b=NB, cc=4),
        in_=io["ce"][:, :, :].rearrange("b (cc p) e -> p b cc e", p=128),
    )

    fw = big.tile([128, NJ * C], F32, tag="fw")
    out_all = big.tile([128, NJ * EMB], F32, tag="out_all")

    # ---------------- per-batch attention ----------------
    ceT_l, ctxT_l, qs_l, ks_l, v_l = {}, {}, {}, {}, {}
    for b in range(NB):
        # transpose ce chunks: (128,64) -> (64,128), 4 chunks -> ceT (64,512)
        ceT_ps = ps_misc.tile([EMB, S], BF, tag="misc")
        for cc in range(4):
            nc.tensor.transpose(
                ceT_ps[:, 128 * cc : 128 * (cc + 1)],
                ce[:, EMB * (4 * b + cc) : EMB * (4 * b + cc + 1)],
                ident[:, :],
            )
        ceT = sb.tile([EMB, S], BF, tag="ceT")
        nc.vector.tensor_copy(ceT[:, :], ceT_ps[:, :])
        ceT_l[b] = ceT

        # ctxT = W_ctx @ ceT + b_ctx   -> (33,512) with ones row
        ctx_ps = ps_misc.tile([ATTN, S], F32, tag="misc")
        nc.tensor.matmul(ctx_ps[:, :], wctxT[:, :], ceT[:, :], start=True, stop=True)
        ctxT = sb.tile([ATTN + 1, S], BF, tag="ctxT")
        nc.vector.tensor_scalar_add(ctxT[0:ATTN, :], ctx_ps[:, :], bctx[:, :])
        nc.vector.memset(ctxT[ATTN : ATTN + 1, :], 1.0)
        ctxT_l[b] = ctxT

        # q/k in spread layout: head h rows 32h..32h+8 (cols zero-padded)
        qs_ps = ps_misc.tile([128, S], F32, tag="misc")
        for h in range(HEADS):
            nc.tensor.matmul(
                qs_ps[32 * h : 32 * (h + 1), :],
                wq_sp[:, 32 * h : 32 * (h + 1)],
                ctxT[0:ATTN, :],
                start=True,
                stop=True,
                tile_position=(0, 32 * h),
            )
        qs = sb.tile([128, S], BF, tag="qs")
        nc.vector.tensor_scalar_add(qs[:, :], qs_ps[:, :], bq_sp[:, :])
        qs_l[b] = qs

        ks_ps = ps_misc.tile([128, S], F32, tag="misc")
        for h in range(HEADS):
            nc.tensor.matmul(
                ks_ps[32 * h : 32 * (h + 1), :],
                wk_sp[:, 32 * h : 32 * (h + 1)],
                ctxT[0:ATTN, :],
                start=True,
                stop=True,
                tile_position=(0, 32 * h),
            )
        ks = sb.tile([128, S], BF, tag="ks")
        nc.vector.tensor_scalar_add(ks[:, :], ks_ps[:, :], bk_sp[:, :])
        ks_l[b] = ks

        # V natural layout per k-chunk, spread: (128,128) head h cols 32h..32h+8
        v_l[b] = []
        for cc in range(4):
            v_ps = ps_misc.tile([128, ATTN], F32, tag="misc")
            nc.tensor.matmul(
                v_ps[:, :],
                ctxT[:, 128 * cc : 128 * (cc + 1)],
                wvT_aug[:, :],
                start=True,
                stop=True,
            )
            v_sp = vsp.tile([128, 128], BF, tag="v_sp")
            nc.vector.memset(v_sp[:, :], 0.0)
            nc.vector.tensor_copy(
                v_sp[:, :].rearrange("p (h g) -> p h g", g=32)[:, :, 0:HD],
                v_ps[:, :].rearrange("p (h d) -> p h d", d=HD),
            )
            v_l[b].append(v_sp)

    # scores^T + exp + AV + sums + output head per batch
    for b in range(NB):
        qs, ks = qs_l[b], ks_l[b]
        expT = [[None, None] for _ in range(4)]
        for cc in range(4):
            for hh in range(2):  # head halves
                sc_ps = ps_scores.tile([128, 2 * S], F32, tag="scores")
                for hi in range(2):
                    h = 2 * hh + hi
                    for jj in range(4):
                        nc.tensor.matmul(
                            sc_ps[32 * jj : 32 * (jj + 1), S * hi : S * (hi + 1)],
                            ks[32 * h : 32 * h + HD, 128 * cc + 32 * jj : 128 * cc + 32 * (jj + 1)],
                            qs[32 * h : 32 * h + HD, :],
                            start=True,
                            stop=True,
                            tile_position=(32 * h, 32 * jj),
                        )
                et = expp.tile([128, 2 * S], BF, tag="expT")
                nc.scalar.activation(et[:, :], sc_ps[:, :], ACTF.Exp, scale=SCALE)
                expT[cc][hh] = et

        avt_ps = ps_av.tile([128, S], F32, tag="avt")
        sums_ps = ps_sum.tile([128, S], F32, tag="sums")
        for cc in range(4):
            for h in range(HEADS):
                mv = expT[cc][h // 2][:, S * (h % 2) : S * (h % 2 + 1)]
                nc.tensor.matmul(
                    avt_ps[32 * h : 32 * (h + 1), :],
                    v_l[b][cc][:, 32 * h : 32 * (h + 1)],
                    mv,
                    start=(cc == 0),
                    stop=(cc == 3),
                    tile_position=(0, 32 * h),
                    skip_group_check=True,
                )
            for h in range(HEADS):
                mv = expT[cc][h // 2][:, S * (h % 2) : S * (h % 2 + 1)]
                nc.tensor.matmul(
                    sums_ps[32 * h : 32 * (h + 1), :],
                    ones_bf[:, :],
                    mv,
                    start=(cc == 0),
                    stop=(cc == 3),
                    tile_position=(0, 32 * h),
                    skip_group_check=True,
                )

        rec = sb.tile([128, S], F32, tag="rec")
        nc.vector.reciprocal(rec[:, :], sums_ps[:, :])
        normT = sb.tile([128, S], BF, tag="normT")
        nc.vector.tensor_tensor(normT[:, :], avt_ps[:, :], rec[:, :], op=ALU.mult)

        # O^T = W_out_spread.T @ normT + b_out  -> (33,512) with ones row
        o_ps = ps_misc.tile([ATTN, S], F32, tag="misc")
        nc.tensor.matmul(o_ps[:, :], wout_sp[:, :], normT[:, :], start=True, stop=True)
        oT = sb.tile([ATTN + 1, S], BF, tag="oT")
        nc.vector.tensor_scalar_add(oT[0:ATTN, :], o_ps[:, :], bout[:, :])
        nc.vector.memset(oT[ATTN : ATTN + 1, :], 1.0)

        # suppression logits z: (128, 16) free = 4*cc + c
        sup_ps = ps_misc.tile([128, 4 * C], F32, tag="misc")
        for cc in range(4):
            nc.tensor.matmul(
                sup_ps[:, C * cc : C * (cc + 1)],
                oT[:, 128 * cc : 128 * (cc + 1)],
                wsupT_aug[:, :],
                start=True,
                stop=True,
            )
        yb = sb.tile([128, 4 * C], F32, tag="yb")
        nc.scalar.activation(yb[:, :], sup_ps[:, :], ACTF.Sigmoid, scale=-1.0)
        # fw = (y + 1) * sharp   (0.5 mean+suppression scale folded into sharp)
        nc.vector.scalar_tensor_tensor(
            fw[:, 16 * b : 16 * (b + 1)],
            yb[:, :],
            1.0,
            sharp[:, 16 * b : 16 * (b + 1)],
            op0=ALU.add,
            op1=ALU.mult,
        )

    # ---------------- final gather-contract ----------------
    for j in range(NJ):
        g = gtiles[j]
        o = out_all[:, EMB * j : EMB * (j + 1)]
        nc.vector.scalar_tensor_tensor(
            o, g[:, 0:EMB], fw[:, C * j : C * j + 1], bemb_bc[:, :],
            op0=ALU.mult, op1=ALU.add,
        )
        for c in range(1, C):
            nc.vector.scalar_tensor_tensor(
                o, g[:, EMB * c : EMB * (c + 1)], fw[:, C * j + c : C * j + c + 1], o,
                op0=ALU.mult, op1=ALU.add,
            )

    # ---------------- store ----------------
    nc.sync.dma_start(
        out=io["out"][:, :, :].rearrange("b (cc p) e -> p b cc e", p=128),
        in_=out_all[:, :].rearrange("p (b cc e) -> p b cc e", b=NB, cc=4),
    )


# ======================= host side =======================

def _prep_weights(inp):
    """Pure layout transforms of the parameters (shared by all cores)."""
    f32 = np.float32

    def bf(x):
        return np.ascontiguousarray(np.asarray(x, f32).astype(BF16))

    W_ctx = np.asarray(inp["W_ctx"], f32)
    W_in = np.asarray(inp["W_in"], f32)
    W_out = np.asarray(inp["W_out"], f32)
    W_sup = np.asarray(inp["W_sup"], f32)
    W_emb = np.asarray(inp["W_emb"], f32)
    b_ctx = np.asarray(inp["b_ctx"], f32)
    b_in = np.asarray(inp["b_in"], f32)
    b_out = np.asarray(inp["b_out"], f32)
    b_sup = np.asarray(inp["b_sup"], f32)
    b_emb = np.asarray(inp["b_emb"], f32)
    temp = np.asarray(inp["temperature"], f32)

    w = {}
    w["wctxT"] = bf(W_ctx.T)                                   # (64,32)
    w["bctx"] = np.ascontiguousarray(b_ctx[:, None])           # (32,1)
    wq = np.zeros((ATTN, 128), f32)
    wk = np.zeros((ATTN, 128), f32)
    bq = np.zeros((128, 1), f32)
    bk = np.zeros((128, 1), f32)
    for h in range(HEADS):
        wq[:, 32 * h : 32 * h + HD] = W_in[HD * h : HD * (h + 1), :].T
        wk[:, 32 * h : 32 * h + HD] = W_in[ATTN + HD * h : ATTN + HD * (h + 1), :].T
        bq[32 * h : 32 * h + HD, 0] = b_in[HD * h : HD * (h + 1)]
        bk[32 * h : 32 * h + HD, 0] = b_in[ATTN + HD * h : ATTN + HD * (h + 1)]
    w["wq_sp"], w["wk_sp"], w["bq_sp"], w["bk_sp"] = bf(wq), bf(wk), bq, bk
    w["wvT_aug"] = bf(np.concatenate([W_in[2 * ATTN :, :].T, b_in[None, 2 * ATTN :]], 0))  # (33,32)
    wout = np.zeros((128, ATTN), f32)
    for h in range(HEADS):
        wout[32 * h : 32 * h + HD, :] = W_out[:, HD * h : HD * (h + 1)].T
    w["wout_sp"] = bf(wout)
    w["bout"] = np.ascontiguousarray(b_out[:, None])
    w["wsupT_aug"] = bf(np.concatenate([W_sup.T, b_sup[None, :]], 0))  # (33,4)
    w["bemb_bc"] = np.ascontiguousarray(np.broadcast_to(b_emb[None, :], (128, EMB)))
    w["temp"] = np.ascontiguousarray(temp[None, :])
    # W3T[q, c*64+e] = W_emb[e, c*Q+q]  (pure transpose)
    w["w3T"] = np.ascontiguousarray(
        W_emb.reshape(EMB, C, Q).transpose(2, 1, 0).reshape(Q, C * EMB)
    )
    return w


def _spec():
    """name -> (shape, mybir dtype) for all per-core DRAM tensors."""
    return {
        "wctxT": ((EMB, ATTN), BF), "bctx": ((ATTN, 1), F32),
        "wq_sp": ((ATTN, 128), BF), "wk_sp": ((ATTN, 128), BF),
        "bq_sp": ((128, 1), F32), "bk_sp": ((128, 1), F32),
        "wvT_aug": ((ATTN + 1, ATTN), BF),
        "wout_sp": ((128, ATTN), BF), "bout": ((ATTN, 1), F32),
        "wsupT_aug": ((ATTN + 1, C), BF),
        "bemb_bc": ((128, EMB), F32), "temp": ((1, HEADS), F32),
        "qidx": ((128, NJ), I32), "rdat": ((128, NJ), I32),
        "w3T": ((Q, C * EMB), F32),
        "ce": ((NB, S, EMB), F32),
    }


def build_bass():
    nc = bacc.Bacc("TRN2", target_bir_lowering=False, debug=False)
    io = {}
    for name, (shape, dt) in _spec().items():
        io[name] = nc.dram_tensor(name, list(shape), dt, kind="ExternalInput").ap()
    io["out"] = nc.dram_tensor("out", [NB, S, EMB], F32, kind="ExternalOutput").ap()
    with tile.TileContext(nc) as tc:
        build_kernel(nc, tc, io)
    nc.compile()
    return nc


def make_in_maps(inputs):
    inp = dict(inputs)
    w = _prep_weights(inp)
    q_idx = np.asarray(inp["q_idx"]).astype(np.int32)
    r_data = np.asarray(inp["r_data"]).astype(np.int32)
    ce = np.asarray(inp["context_embedding"], np.float32)

    in_maps = []
    for k in range(NCORES):
        m = dict(w)
        qs = q_idx[NB * k : NB * (k + 1)]          # (4,512)
        rs = r_data[NB * k : NB * (k + 1)]
        # token-tile layout: [p, j] with j = 4*b + cc, s = 128*cc + p
        m["qidx"] = np.ascontiguousarray(
            qs.reshape(NB, 4, 128).transpose(2, 0, 1).reshape(128, NJ)
        )
        m["rdat"] = np.ascontiguousarray(
            rs.reshape(NB, 4, 128).transpose(2, 0, 1).reshape(128, NJ)
        )
        m["ce"] = np.ascontiguousarray(ce[NB * k : NB * (k + 1)])
        in_maps.append(m)
    return in_maps


_NC_CACHE = {}


def kernel(**inputs) -> np.ndarray:
    if "nc" not in _NC_CACHE:
        _NC_CACHE["nc"] = build_bass()
    nc = _NC_CACHE["nc"]
    in_maps = make_in_maps(inputs)
    res = run_bass_kernel_spmd(nc, in_maps, core_ids=list(range(NCORES)))
    out = np.concatenate([res.results[k]["out"] for k in range(NCORES)], axis=0)
    return out.astype(np.float32)
